# revision 1
# baseline (speedup 1.0000x reference)
"""Multi-head attention (q/k/v projections + softmax attention + out-projection)
on 8 Trainium2 NeuronCores.

Sharding: 16 (batch, head) units over 8 cores -> core c handles batch n = c//4
and head pair hp = c%4 (columns 128*hp : 128*hp+128 of the projections).
Per-core partial outputs (each pair's contribution to mix @ Wo) are summed on
host per batch, + bo.

Device kernel (per core):
  - Host pre-transposes q[n],k[n],v[n] -> xT [512, 4096] so the D-contraction
    projections need no on-device transpose.
  - All matmuls run in float32r (TF32-like, 1 cycle/row vs 4 for fp32 on the
    PE; measured end-to-end |err|_max/|out|_max ~ 5e-4 vs fp32's 3e-6).
  - QPT/KPT [128, 512]x8 chunk tiles: W.T @ x with head-dim on partitions;
    q scaled by 1/8, biases folded in via DVE tensor_scalar(mult, add).
  - VP chunk tiles [128lkv, 4, 130]: v-projection computed un-transposed (lkv
    on partitions) with the bias added via an extra K=1 ones x bv matmul row;
    layout per j: [h0 c(64) | ones | h1 c(64) | ones] - the ones column makes
    each PV matmul also accumulate sum(exp) into psum row 64.
  - Attention in S^T orientation: S^T[lkv,lq] = KPT_h.T @ QPT_h (K=64, heads
    at PE row groups 0/64), exp on ScalarE with FD=1024 tiles (no max
    subtraction needed: scores ~ N(0,1), exp range is tiny), PV accumulates
    mixT[c,lq] + sumexp transpose-free. ScalarE exp (~270us busy) is the
    critical engine; S/PV matmuls (~220us) hide under it.
  - sumexp [1,1024] is transposed to [128,8] partition-major via a DRAM
    bounce, reciprocal on DVE, and the normalization is folded into the
    out-projection: per-head out-proj psums scaled per-partition (lq) by
    1/sumexp on DVE, then summed on GPSIMD.
  - One unified 8-bank PSUM pool (st0/st1/pv0/pv1 tags x 2 banks); the
    projection phase rotates through the same tags.
"""

import numpy as np

import concourse.bacc as bacc
import concourse.mybir as mybir
import concourse.tile as tile
from concourse import bass_utils

P = 128
L = 4096
D = 512
F32 = mybir.dt.float32
F32R_DT = mybir.dt.float32r
AF = mybir.ActivationFunctionType

_NC = None
F32R = True  # run matmuls in float32r (TF32-like, 4x faster than fp32 on PE)


def _mm(nc, out, lhsT, rhs, f32r=True, **kw):
    nc.tensor.matmul(out, lhsT=lhsT, rhs=rhs, **kw)


def build():
    nc = bacc.Bacc("TRN2", target_bir_lowering=False, debug=False)

    xqt = nc.dram_tensor("xqt", (D, L), F32R_DT, kind="ExternalInput").ap()
    xkt = nc.dram_tensor("xkt", (D, L), F32R_DT, kind="ExternalInput").ap()
    xvt = nc.dram_tensor("xvt", (D, L), F32R_DT, kind="ExternalInput").ap()
    wq = nc.dram_tensor("wq", (D, P), F32R_DT, kind="ExternalInput").ap()
    wk = nc.dram_tensor("wk", (D, P), F32R_DT, kind="ExternalInput").ap()
    wv = nc.dram_tensor("wv", (D, P), F32R_DT, kind="ExternalInput").ap()
    wo = nc.dram_tensor("wo", (P, D), F32R_DT, kind="ExternalInput").ap()
    bqs = nc.dram_tensor("bqs", (P, 1), F32, kind="ExternalInput").ap()
    bkc = nc.dram_tensor("bkc", (P, 1), F32, kind="ExternalInput").ap()
    bvr = nc.dram_tensor("bvr", (1, P), F32R_DT, kind="ExternalInput").ap()
    out = nc.dram_tensor("out", (L, D), F32, kind="ExternalOutput").ap()

    with tile.TileContext(nc) as tc:
        with tc.tile_pool(name="const", bufs=1) as const, \
             tc.tile_pool(name="persist", bufs=1) as persist:
            wq_sb = const.tile([P, 4, P], F32R_DT, tag="wq")
            nc.sync.dma_start(wq_sb, wq.rearrange("(o p) m -> p o m", p=P))
            wk_sb = const.tile([P, 4, P], F32R_DT, tag="wk")
            nc.sync.dma_start(wk_sb, wk.rearrange("(o p) m -> p o m", p=P))
            wv_sb = const.tile([P, 4, P], F32R_DT, tag="wv")
            nc.sync.dma_start(wv_sb, wv.rearrange("(o p) m -> p o m", p=P))
            wo_sb = const.tile([P, D], F32R_DT, tag="wo")
            nc.sync.dma_start(wo_sb, wo)
            bq_sb = const.tile([P, 1], F32, tag="bq")
            nc.sync.dma_start(bq_sb, bqs)
            bk_sb = const.tile([P, 1], F32, tag="bk")
            nc.sync.dma_start(bk_sb, bkc)
            bvr_sb = const.tile([1, P], F32R_DT, tag="bvr")
            nc.sync.dma_start(bvr_sb, bvr)
            onesr = const.tile([1, P], F32R_DT, tag="onesr")
            nc.scalar.activation(onesr, bvr_sb, AF.Identity,
                                 bias=1.0, scale=0.0)

            qpt_t = [persist.tile([P, 512], F32R_DT, tag=f"qpt{c}",
                                  name=f"qpt{c}") for c in range(8)]
            kpt_t = [persist.tile([P, 512], F32R_DT, tag=f"kpt{c}",
                                  name=f"kpt{c}") for c in range(8)]
            vp_t = [persist.tile([P, 4, 130], F32R_DT, tag=f"vp{c}",
                                 name=f"vp{c}") for c in range(8)]
            ones_in = bq_sb[:, :, None].to_broadcast((P, 4, 1))
            for c in range(8):
                nc.scalar.activation(vp_t[c][:, :, 64:65], ones_in,
                                     AF.Identity, bias=1.0, scale=0.0)
                nc.scalar.activation(vp_t[c][:, :, 129:130], ones_in,
                                     AF.Identity, bias=1.0, scale=0.0)

            # ---------------- projections + attention ----------------
            # One PSUM pool for everything (8 banks exactly): st0/st1 and
            # pv0/pv1 tags, [128,1024] fp32 = 2 banks each. Projections
            # rotate through the same tags so attention tiles never wait on
            # a disjoint pool's address range.
            with tc.tile_pool(name="xs", bufs=2) as xs, \
                 tc.tile_pool(name="psp", bufs=1, space="PSUM") as psp, \
                 tc.tile_pool(name="esp", bufs=4) as esp, \
                 tc.tile_pool(name="smallp", bufs=4) as smallp, \
                 tc.tile_pool(name="mixp", bufs=4) as mixp, \
                 tc.tile_pool(name="outp", bufs=3) as outp, \
                 tc.tile_pool(name="dramp", bufs=2, space="DRAM") as dramp:
                xqv = xqt.rearrange("(o p) l -> p o l", p=P)
                xkv = xkt.rearrange("(o p) l -> p o l", p=P)
                xvv = xvt.rearrange("(o p) l -> p o l", p=P)
                for ch in range(8):
                    sl = slice(ch * 512, (ch + 1) * 512)
                    # K chunk
                    xtk = xs.tile([P, 4, 512], F32R_DT, tag="xtk")
                    nc.sync.dma_start(xtk, xkv[:, :, sl])
                    ps = psp.tile([P, 512], F32, tag="st0", name="kps")
                    for dk in range(4):
                        _mm(nc, ps, wk_sb[:, dk, :], xtk[:, dk, :],
                            start=(dk == 0), stop=(dk == 3))
                    nc.vector.tensor_scalar(
                        kpt_t[ch][:], ps, 1.0, bk_sb,
                        mybir.AluOpType.mult, mybir.AluOpType.add)
                    # V chunk
                    xtv = xs.tile([P, 4, 512], F32R_DT, tag="xtv")
                    nc.gpsimd.dma_start(xtv, xvv[:, :, sl])
                    for js in range(4):
                        j = ch * 4 + js
                        psv = psp.tile([P, P], F32, tag=f"pv{js % 2}",
                                       name="psv")
                        for dk in range(4):
                            _mm(nc, psv, xtv[:, dk, js * P:(js + 1) * P],
                                wv_sb[:, dk, :],
                                start=(dk == 0), stop=False)
                        _mm(nc, psv, onesr, bvr_sb,
                            start=False, stop=True)
                        nc.vector.tensor_copy(vp_t[ch][:, js, 0:64],
                                               psv[:, 0:64])
                        nc.vector.tensor_copy(vp_t[ch][:, js, 65:129],
                                              psv[:, 64:128])
                    # Q chunk
                    xtq = xs.tile([P, 4, 512], F32R_DT, tag="xtq")
                    nc.gpsimd.dma_start(xtq, xqv[:, :, sl])
                    psq = psp.tile([P, 512], F32, tag="st1", name="qps")
                    for dk in range(4):
                        _mm(nc, psq, wq_sb[:, dk, :], xtq[:, dk, :],
                            start=(dk == 0), stop=(dk == 3))
                    nc.vector.tensor_scalar(
                        qpt_t[ch][:], psq, 0.125, bq_sb,
                        mybir.AluOpType.mult, mybir.AluOpType.add)

                # ---------------- attention ----------------
                for lqc in range(4):
                    q0 = lqc * 1024
                    pv_ps = [psp.tile([P, 1024], F32, tag=f"pv{h}", name=f"pv{h}")
                             for h in range(2)]
                    for j in range(32):
                        for h in range(2):
                            hb = h * 64
                            st = psp.tile([P, 1024], F32, tag=f"st{h}")
                            for hf in range(2):
                                _mm(nc, st[:, hf * 512:(hf + 1) * 512],
                                    kpt_t[j // 4][hb:hb + 64,
                                                  (j % 4) * P:(j % 4 + 1) * P],
                                    qpt_t[2 * lqc + hf][hb:hb + 64, :],
                                    start=True, stop=True)
                            est = esp.tile([P, 1024], F32R_DT, tag=f"est{h}")
                            nc.scalar.activation(est, st, AF.Exp)
                            for hf in range(2):
                                _mm(nc, pv_ps[h][0:65, hf * 512:(hf + 1) * 512],
                                    vp_t[j // 4][:, j % 4, h * 65:(h + 1) * 65],
                                    est[:, hf * 512:(hf + 1) * 512],
                                    start=(j == 0), stop=(j == 31))
                    recips = []
                    mix2 = mixp.tile([P, 1024], F32R_DT, tag="mix2")
                    for h in range(2):
                        row = smallp.tile([1, 1024], F32, tag=f"row{h}")
                        nc.vector.tensor_copy(row, pv_ps[h][64:65, :])
                        drow = dramp.tile([1024], F32, tag=f"drow{h}")
                        nc.sync.dma_start(drow, row)
                        sumsT = smallp.tile([P, 8], F32, tag=f"sT{h}")
                        # sumsT[p, s] = sums[s*128 + p]
                        nc.sync.dma_start(
                            sumsT, drow.rearrange("(s p) -> p s", p=P))
                        rT = smallp.tile([P, 8], F32, tag=f"rT{h}")
                        nc.vector.reciprocal(rT, sumsT)
                        recips.append(rT)
                        nc.vector.tensor_copy(mix2[h * 64:(h + 1) * 64, :],
                                              pv_ps[h][0:64, :])
                    for s in range(8):
                        ops = [psp.tile([P, D], F32, tag=f"pv{h}", name=f"op{h}")
                               for h in range(2)]
                        for h in range(2):
                            _mm(nc, ops[h],
                                mix2[h * 64:(h + 1) * 64, s * P:(s + 1) * P],
                                wo_sb[h * 64:(h + 1) * 64, :],
                                start=True, stop=True)
                        t0 = outp.tile([P, D], F32, tag="t0")
                        nc.vector.tensor_scalar_mul(t0, ops[0],
                                                    recips[0][:, s:s + 1])
                        t1 = outp.tile([P, D], F32, tag="t1")
                        nc.vector.tensor_scalar_mul(t1, ops[1],
                                                    recips[1][:, s:s + 1])
                        ob = outp.tile([P, D], F32, tag="ob")
                        nc.gpsimd.tensor_add(ob, t0, t1)
                        nc.sync.dma_start(
                            out[q0 + s * P:q0 + (s + 1) * P, :], ob)

    nc.compile()
    return nc


def get_nc():
    global _NC
    if _NC is None:
        _NC = build()
    return _NC


def make_in_maps(q, k, v, Wq, bq, Wk, bk, Wv, bv, Wo, bo):
    q = np.asarray(q, np.float32)
    k = np.asarray(k, np.float32)
    v = np.asarray(v, np.float32)
    Wq = np.asarray(Wq, np.float32)
    Wk = np.asarray(Wk, np.float32)
    Wv = np.asarray(Wv, np.float32)
    Wo = np.asarray(Wo, np.float32)
    bq = np.asarray(bq, np.float32)
    bk = np.asarray(bk, np.float32)
    bv = np.asarray(bv, np.float32)
    xts = {}
    for n in range(2):
        xts[n] = (np.ascontiguousarray(q[n].T),
                  np.ascontiguousarray(k[n].T),
                  np.ascontiguousarray(v[n].T))
    in_maps = []
    for c in range(8):
        n, hp = c // 4, c % 4
        sl = slice(P * hp, P * (hp + 1))
        xq, xk, xv = xts[n]
        in_maps.append({
            "xqt": xq, "xkt": xk, "xvt": xv,
            "wq": np.ascontiguousarray(Wq[:, sl]),
            "wk": np.ascontiguousarray(Wk[:, sl]),
            "wv": np.ascontiguousarray(Wv[:, sl]),
            "wo": np.ascontiguousarray(Wo[sl, :]),
            "bqs": (bq[sl] * 0.125).reshape(P, 1).astype(np.float32),
            "bkc": bk[sl].reshape(P, 1).astype(np.float32),
            "bvr": bv[sl].reshape(1, P).astype(np.float32),
        })
    return in_maps


def assemble(results, bo):
    bo = np.asarray(bo, np.float32)
    out = np.zeros((2, L, D), np.float32)
    for c in range(8):
        out[c // 4] += results[c]["out"]
    out += bo[None, None, :]
    return out


def kernel(q, k, v, Wq, bq, Wk, bk, Wv, bv, Wo, bo):
    nc = get_nc()
    in_maps = make_in_maps(q, k, v, Wq, bq, Wk, bk, Wv, bv, Wo, bo)
    res = bass_utils.run_bass_kernel_spmd(nc, in_maps, core_ids=list(range(8)))
    return assemble(res.results, bo)


if __name__ == "__main__":
    build()
    print("build ok")



# revision 32
# speedup vs baseline: 1.4632x; 1.4632x over previous
"""Multi-head attention (q/k/v projections + softmax attention + out-projection)
on 8 Trainium2 NeuronCores.

Sharding: 16 (batch, head-pair) units over 8 cores -> core c handles batch
n = c//4 and head pair hp = c%4 (columns 128*hp : 128*hp+128 of the
projections).  Per-core partial outputs are summed on host per batch, + bo.

Device kernel (v2) per core:
  - Host pre-transposes q[n],k[n],v[n] -> xT [512, 4096] in bf16; weights in
    bf16 (wq/bq pre-scaled by 1/8).  Projections run in bf16 (1 cyc/row).
  - QPT/KPT [128, 512]x8 f32r tiles (head-dim on partitions): W.T @ x, biases
    folded via DVE tensor_scalar.  VP [128lkv, 4js, 2h, 65] bf16 tiles with a
    ones column (col 64) used to accumulate sum(exp) into the mix psum.
  - Attention in S^T orientation: S^T[kv,lq] = KPT_h.T @ QPT_h (f32r, K=64),
    into st psum [128,1024].  exp is SPLIT between ScalarE (activation Exp ->
    bf16 est) and DVE (Schraudolph bit-trick: int16(A*x+B) bitcast as bf16,
    ~3% rel err) on a fixed tile schedule to balance engine load.
  - PV with est STATIONARY: mix[q,c] psum slots [128, 65] = est_chunk.T @ vp
    (bf16, 65 rows/matmul, full 128x128 PE utilization; 2x fewer PE cycles
    than the vp-stationary orientation).  16 slots (2 heads x 8 q-chunks)
    packed 6-per-bank into 3 psum banks.
  - Normalization folded early: recip(sumexp) per-partition (q) -> DVE
    tensor_scalar_mul -> normalized mix bf16; PE-transposed (via identity)
    into a psum bank; both heads stacked -> one [128,128] mixT; out-proj is a
    single matmul [128q, 512] per q-chunk accumulating both heads.
  - Output written in bf16 (halves the out DMA); host sums partials + bo.
  - PSUM: st0(2 banks) st1(2) mix(3) trp(1).  Drain work for chunk L is
    interleaved into chunk L+1's j-loop emission.
"""

import numpy as np
import ml_dtypes

import concourse.bacc as bacc
import concourse.mybir as mybir
import concourse.tile as tile
from concourse import bass_utils

P = 128
L = 4096
D = 512
F32 = mybir.dt.float32
F32R = mybir.dt.float32r
BF16 = mybir.dt.bfloat16
I16 = mybir.dt.int16
AF = mybir.ActivationFunctionType
ALU = mybir.AluOpType

# Schraudolph exp-to-bf16 constants: bf16(int16(A16*x + B16)) ~ exp(x)
A16 = (1 << 7) / float(np.log(2.0))
B16 = 16256.0 - 4.5

# exp tile (j, h) goes to DVE (Schraudolph) on a spread schedule: at most
# one of the two heads of a given j, never in the first 4 j's of a chunk
# (there DVE absorbs the drain burst of the previous chunk instead).
DVE_MOD, DVE_K = 4, 3
DVE_JMIN = 4

_NC = None

# bisection flags (timing experiments only; results are wrong when set)
PROJ_ONLY = False
SKIP_PV = False
SKIP_EXP = False
SKIP_DRAIN = False
ALL_SCALAR_EXP = False
PROJ_UPFRONT = False


def _dve_tile(j, h):
    """DVE (Schraudolph side-channel) tiles: mid-chunk only (the trp psum
    bank they use hosts drain work near chunk boundaries)."""
    if ALL_SCALAR_EXP:
        return False
    return 8 <= j <= 30 and (j % 3) != 1 and h == (j % 2)


def _jh_seq():
    """(lqc, j, h) in emission order."""
    for lqc in range(4):
        for j in range(32):
            for h in range(2):
                yield (lqc, j, h)


def _mix_slot(t):
    """(bank, f32-element offset) of mix psum slot t (6 per 2KB bank)."""
    return t // 6, (t % 6) * 65


def build():
    nc = bacc.Bacc("TRN2", target_bir_lowering=False, debug=False)

    xqt = nc.dram_tensor("xqt", (D, L), BF16, kind="ExternalInput").ap()
    xkt = nc.dram_tensor("xkt", (D, L), BF16, kind="ExternalInput").ap()
    xvt = nc.dram_tensor("xvt", (D, L), BF16, kind="ExternalInput").ap()
    wq = nc.dram_tensor("wq", (D, P), BF16, kind="ExternalInput").ap()
    wk = nc.dram_tensor("wk", (D, P), BF16, kind="ExternalInput").ap()
    wv = nc.dram_tensor("wv", (D, P), BF16, kind="ExternalInput").ap()
    wo = nc.dram_tensor("wo", (P, D), BF16, kind="ExternalInput").ap()
    bqs = nc.dram_tensor("bqs", (P, 1), F32, kind="ExternalInput").ap()
    bkc = nc.dram_tensor("bkc", (P, 1), F32, kind="ExternalInput").ap()
    bvr = nc.dram_tensor("bvr", (1, P), BF16, kind="ExternalInput").ap()
    ident = nc.dram_tensor("ident", (P, P), BF16, kind="ExternalInput").ap()
    out = nc.dram_tensor("out", (L, D), BF16, kind="ExternalOutput").ap()

    with tile.TileContext(nc) as tc:
        with tc.tile_pool(name="const", bufs=1) as const, \
             tc.tile_pool(name="persist", bufs=1) as persist, \
             tc.tile_pool(name="psp", bufs=1, space="PSUM") as psp, \
             tc.tile_pool(name="xs", bufs=3) as xs, \
             tc.tile_pool(name="esp", bufs=4) as esp, \
             tc.tile_pool(name="smallp", bufs=6) as smallp, \
             tc.tile_pool(name="mixnp", bufs=6) as mixnp, \
             tc.tile_pool(name="mixtp", bufs=8) as mixtp, \
             tc.tile_pool(name="outp", bufs=3) as outp:
            # ---- psum layout (created in order: banks 0-1,2-3,4-6,7) ----
            st_t = [psp.tile([P, 1024], F32, tag="st0", name="st0"),
                    psp.tile([P, 1024], F32, tag="st1", name="st1")]
            mix_ps = [psp.tile([P, 512], F32, tag=f"mixb{b}",
                               name=f"mixb{b}") for b in range(3)]
            trp_ps = psp.tile([P, 512], F32, tag="trp", name="trp")
            trp_bf = trp_ps.bitcast(BF16)  # [P, 1024] bf16 view

            # ---- constants (K-path first; the rest ride behind the
            # first data chunks on the serial DMA channel) ----
            wk_sb = const.tile([P, 4, P], BF16, tag="wk")
            nc.sync.dma_start(wk_sb, wk.rearrange("(o p) m -> p o m", p=P))
            wq_sb = const.tile([P, 4, P], BF16, tag="wq")
            nc.sync.dma_start(wq_sb, wq.rearrange("(o p) m -> p o m", p=P))
            bq_sb = const.tile([P, 1], F32, tag="bq")
            nc.gpsimd.dma_start(bq_sb, bqs)
            bk_sb = const.tile([P, 1], F32, tag="bk")
            nc.gpsimd.dma_start(bk_sb, bkc)
            wv_sb = const.tile([P, 4, P], BF16, tag="wv")
            nc.gpsimd.dma_start(wv_sb, wv.rearrange("(o p) m -> p o m", p=P))
            bvr_sb = const.tile([1, P], BF16, tag="bvr")
            nc.gpsimd.dma_start(bvr_sb, bvr)
            wo_sb = const.tile([P, D], BF16, tag="wo")
            nc.scalar.dma_start(wo_sb, wo)
            id_sb = const.tile([P, P], BF16, tag="ident")
            nc.scalar.dma_start(id_sb, ident)
            onesr = const.tile([1, P], BF16, tag="onesr")
            nc.vector.memset(onesr, 1.0)

            qpt_t = [persist.tile([P, 512], F32R, tag=f"qpt{c}",
                                  name=f"qpt{c}") for c in range(8)]
            kpt_t = [persist.tile([P, 512], F32R, tag=f"kpt{c}",
                                  name=f"kpt{c}") for c in range(8)]
            vp_t = [persist.tile([P, 4, 2, 65], BF16, tag=f"vp{c}",
                                 name=f"vp{c}") for c in range(8)]
            for c in range(8):
                nc.vector.memset(vp_t[c][:, :, :, 64:65], 1.0)

            # ---------------- projection pieces ----------------
            # Projections for chunk 0 (and q of chunk 1) run up front; the
            # rest interleave into chunk-0's j-loop (the serial DMA channel
            # is the projection-phase bottleneck otherwise).  Proj matmuls
            # borrow st psum half-banks; Tile WAR deps serialize them into
            # the S/exp ping-pong safely.
            xqv = xqt.rearrange("(o p) l -> p o l", p=P)
            xkv = xkt.rearrange("(o p) l -> p o l", p=P)
            xvv = xvt.rearrange("(o p) l -> p o l", p=P)
            chunk_bufs = {}

            def dma_chunk_kq(ch):
                sl = slice(ch * 512, (ch + 1) * 512)
                xtk = xs.tile([P, 4, 512], BF16, tag="xtk")
                nc.sync.dma_start(xtk, xkv[:, :, sl])
                xtq = xs.tile([P, 4, 512], BF16, tag="xtq")
                nc.scalar.dma_start(xtq, xqv[:, :, sl])
                chunk_bufs[ch] = [xtk, None, xtq]

            def dma_chunk_v(ch):
                sl = slice(ch * 512, (ch + 1) * 512)
                xtv = xs.tile([P, 4, 512], BF16, tag="xtv")
                nc.gpsimd.dma_start(xtv, xvv[:, :, sl])
                chunk_bufs[ch][1] = xtv

            def dma_chunk(ch):
                dma_chunk_kq(ch)
                dma_chunk_v(ch)

            def proj_k(ch, par=1):
                xtk = chunk_bufs[ch][0]
                psk = st_t[par][:, 0:512]
                for dk in range(4):
                    nc.tensor.matmul(psk, lhsT=wk_sb[:, dk, :],
                                     rhs=xtk[:, dk, :],
                                     start=(dk == 0), stop=(dk == 3))
                nc.vector.tensor_scalar(kpt_t[ch][:], psk, 1.0, bk_sb,
                                        ALU.mult, ALU.add)

            def proj_v1(ch, js, par=None):
                xtv = chunk_bufs[ch][1]
                psv = st_t[js % 2 if par is None else par][:, 512:640]
                for dk in range(4):
                    nc.tensor.matmul(psv,
                                     lhsT=xtv[:, dk, js * P:(js + 1) * P],
                                     rhs=wv_sb[:, dk, :],
                                     start=(dk == 0), stop=False)
                nc.tensor.matmul(psv, lhsT=onesr, rhs=bvr_sb,
                                 start=False, stop=True)
                for h in range(2):
                    nc.vector.tensor_copy(vp_t[ch][:, js, h, 0:64],
                                          psv[:, h * 64:(h + 1) * 64])

            def proj_v(ch, js0):
                proj_v1(ch, js0)
                proj_v1(ch, js0 + 1)

            def proj_q(ch, par=0):
                xtq = chunk_bufs[ch][2]
                psq = st_t[par][:, 0:512]
                for dk in range(4):
                    nc.tensor.matmul(psq, lhsT=wq_sb[:, dk, :],
                                     rhs=xtq[:, dk, :],
                                     start=(dk == 0), stop=(dk == 3))
                nc.vector.tensor_scalar(qpt_t[ch][:], psq, 1.0, bq_sb,
                                        ALU.mult, ALU.add)

            # prefix ordered for the serial DMA channel: K+Q paths of
            # chunks 0/1 first (the first S needs kpt[0], qpt[0], qpt[1]),
            # V data afterwards (first PV tolerates a late vp[0]).
            dma_chunk_kq(0)
            dma_chunk_kq(1)
            proj_k(0)
            proj_q(0)
            proj_q(1)
            dma_chunk_v(0)
            proj_v(0, 0)
            proj_v(0, 2)
            dma_chunk_v(1)
            # remaining pieces scheduled into chunk-0 j-loop tile slots:
            # chunk c ready before first use at tile 8c.
            # pieces take the st parity of the slot they pop at; "dma"
            # items carry no st dependence and emit immediately.
            proj_work = {}
            proj_work[0] = [lambda par: proj_k(1, par),
                            lambda par: proj_v1(1, 0, par)]
            proj_work[1] = [lambda par: proj_v1(1, 1, par),
                            lambda par: proj_v1(1, 2, par)]
            proj_work[2] = [lambda par: proj_v1(1, 3, par)]
            for c in range(2, 8):
                base = 6 * (c - 1)
                proj_work.setdefault(max(0, base - 5), []).append(
                    lambda par, c=c: dma_chunk(c))
                proj_work.setdefault(base + 0, []).append(
                    lambda par, c=c: proj_k(c, par))
                proj_work.setdefault(base + 1, []).append(
                    lambda par, c=c: proj_v1(c, 0, par))
                proj_work.setdefault(base + 2, []).append(
                    lambda par, c=c: proj_v1(c, 1, par))
                proj_work.setdefault(base + 3, []).append(
                    lambda par, c=c: proj_v1(c, 2, par))
                proj_work.setdefault(base + 4, []).append(
                    lambda par, c=c: proj_v1(c, 3, par))
                proj_work.setdefault(base + 5, []).append(
                    lambda par, c=c: proj_q(c, par))
            if PROJ_ONLY or PROJ_UPFRONT:
                for i in sorted(proj_work):
                    for w in proj_work[i]:
                        w(None)
                proj_work = {}

            # ---------------- attention ----------------
            def drain_phase_a(lqc):
                """Read out mix psum: recip+normalize+transpose+gather.
                Must be emitted BEFORE the next chunk's PV writes to mix
                (Tile dependency order is emission order).  Returns the 8
                gathered mixT tiles."""
                mixts = []
                for s in range(8):
                    mixn = [None, None]
                    for h in range(2):
                        t = h * 8 + s
                        b, off = _mix_slot(t)
                        rT = smallp.tile([P, 1], F32, tag=f"rT{t % 4}")
                        nc.vector.reciprocal(
                            rT, mix_ps[b][:, off + 64:off + 65])
                        mn = mixnp.tile([P, 64], BF16,
                                        tag=f"mn{(2 * s + h) % 6}")
                        nc.vector.tensor_scalar_mul(
                            mn, mix_ps[b][:, off:off + 64], rT)
                        mixn[h] = mn
                    for h in range(2):
                        nc.tensor.transpose(
                            trp_bf[h * 64:(h + 1) * 64, s * P:(s + 1) * P],
                            mixn[h], id_sb)
                    mt = mixtp.tile([P, P], BF16, tag=f"mt{s}")
                    nc.vector.tensor_copy(mt, trp_bf[:, s * P:(s + 1) * P])
                    mixts.append(mt)
                return mixts

            def drain_phase_b(lqc, mixts, last=False):
                """Out-projection closures.  Normally serialized on the trp
                bank (they interleave into the next chunk's j-loop); for the
                last chunk the freed mix banks give a 3-way rotation so the
                tail drains fast."""
                q0 = lqc * 1024

                def item(s):
                    def emit():
                        op = mix_ps[s % 3] if last else trp_ps
                        nc.tensor.matmul(op, lhsT=mixts[s], rhs=wo_sb,
                                         start=True, stop=True)
                        ob = outp.tile([P, D], BF16, tag=f"ob{s % 3}")
                        nc.vector.tensor_copy(ob, op)
                        nc.sync.dma_start(
                            out[q0 + s * P:q0 + (s + 1) * P, :], ob)
                    return emit
                return [item(s) for s in range(8)]

            def emit_s(lqc, j, h, sti):
                """S^T matmuls for tile (lqc, j, h) into st_t[sti]."""
                jc, jj = j // 4, j % 4
                hb = h * 64
                st = st_t[sti]
                for hf in range(2):
                    nc.tensor.matmul(
                        st[:, hf * 512:(hf + 1) * 512],
                        lhsT=kpt_t[jc][hb:hb + 64, jj * P:(jj + 1) * P],
                        rhs=qpt_t[2 * lqc + hf][hb:hb + 64, :],
                        start=True, stop=True)

            def emit_pv(lqc, j, h, est):
                jc, jj = j // 4, j % 4
                vrhs = vp_t[jc][:, jj, h, :]
                for s in range(8 if not SKIP_PV else 0):
                    t = h * 8 + s
                    b, off = _mix_slot(t)
                    start = (j == 0) and (t == b * 6)
                    stop = (j == 31) and (t == min(b * 6 + 5, 15))
                    nc.tensor.matmul(
                        mix_ps[b][:, off:off + 65],
                        lhsT=est[:, s * P:(s + 1) * P], rhs=vrhs,
                        start=start, stop=stop)

            def queue_dve(lqc, j, h):
                """Schraudolph side-channel through the trp bank: two
                [128,512] S-halves, each exp'd on DVE.  Keeps the main
                st0/st1 ping-pong purely ScalarE-fed."""
                jc, jj = j // 4, j % 4
                hb = h * 64
                est = esp.tile([P, 1024], BF16, tag="estd")
                esti = est.bitcast(I16)

                def stage_s(hf):
                    def emit():
                        nc.tensor.matmul(
                            trp_ps,
                            lhsT=kpt_t[jc][hb:hb + 64, jj * P:(jj + 1) * P],
                            rhs=qpt_t[2 * lqc + hf][hb:hb + 64, :],
                            start=True, stop=True)
                    return emit

                def stage_e(hf):
                    def emit():
                        nc.vector.tensor_scalar(
                            esti[:, hf * 512:(hf + 1) * 512], trp_ps,
                            A16, B16, ALU.mult, ALU.add)
                    return emit

                return [stage_s(0), stage_e(0), stage_s(1), stage_e(1),
                        lambda: emit_pv(lqc, j, h, est)]

            tiles = list(_jh_seq())
            if PROJ_ONLY:
                tiles = []
            scal_tiles = [t for t in tiles
                          if SKIP_EXP or not _dve_tile(t[1], t[2])]
            pending = []
            dve_q = []
            proj_qs = []
            # software pipeline: S runs one tile-slot ahead of exp/PV per
            # parity so the PE keeps st full while ScalarE exps.
            si = 0  # next scal tile to process
            if len(scal_tiles) >= 2:
                emit_s(*scal_tiles[0], 0)
                emit_s(*scal_tiles[1], 1)
            for i, (lqc, j, h) in enumerate(tiles):
                if j == 31 and h == 0:
                    # dve-queued PVs must land before the j=31 stop flags
                    while dve_q:
                        dve_q.pop(0)()
                proj_qs.extend(proj_work.pop(i, []))
                if not SKIP_EXP and _dve_tile(j, h):
                    dve_q.extend(queue_dve(lqc, j, h))
                    for _ in range(2):
                        if dve_q:
                            dve_q.pop(0)()
                    if pending and not dve_q:
                        pending.pop(0)()
                else:
                    par = si % 2
                    st = st_t[par]
                    est = esp.tile([P, 1024], BF16, tag=f"est{par}")
                    if not SKIP_EXP:
                        nc.scalar.activation(est, st, AF.Exp)
                    # st_t[par] is free between exp(i) and S(i+2): proj
                    # pieces borrowing this parity's half-banks must be
                    # emitted exactly here (a piece of the other parity
                    # would corrupt an in-flight S -> exp pair).
                    for _ in range(2):
                        if dve_q:
                            dve_q.pop(0)()
                    for _ in range(2):
                        if proj_qs:
                            proj_qs.pop(0)(par)
                    if pending and not dve_q:
                        pending.pop(0)()
                    if si + 2 < len(scal_tiles):
                        emit_s(*scal_tiles[si + 2], par)
                    emit_pv(lqc, j, h, est)
                    si += 1
                if j == 31 and h == 1:
                    while dve_q:
                        dve_q.pop(0)()
                    while pending:
                        pending.pop(0)()
                    if not (SKIP_DRAIN or SKIP_PV):
                        mixts = drain_phase_a(lqc)
                        pending = drain_phase_b(lqc, mixts, last=(lqc == 3))
            while pending:
                pending.pop(0)()

    nc.compile()
    return nc


def get_nc():
    global _NC
    if _NC is None:
        _NC = build()
    return _NC


def make_in_maps(q, k, v, Wq, bq, Wk, bk, Wv, bv, Wo, bo):
    bf = ml_dtypes.bfloat16
    q = np.asarray(q, np.float32)
    k = np.asarray(k, np.float32)
    v = np.asarray(v, np.float32)
    Wq = np.asarray(Wq, np.float32)
    Wk = np.asarray(Wk, np.float32)
    Wv = np.asarray(Wv, np.float32)
    Wo = np.asarray(Wo, np.float32)
    bq = np.asarray(bq, np.float32)
    bk = np.asarray(bk, np.float32)
    bv = np.asarray(bv, np.float32)
    identity = np.eye(P, dtype=bf)
    xts = {}
    for n in range(2):
        xts[n] = (np.ascontiguousarray(q[n].T).astype(bf),
                  np.ascontiguousarray(k[n].T).astype(bf),
                  np.ascontiguousarray(v[n].T).astype(bf))
    in_maps = []
    for c in range(8):
        n, hp = c // 4, c % 4
        sl = slice(P * hp, P * (hp + 1))
        xq, xk, xv = xts[n]
        in_maps.append({
            "xqt": xq, "xkt": xk, "xvt": xv,
            "wq": np.ascontiguousarray(Wq[:, sl] * 0.125).astype(bf),
            "wk": np.ascontiguousarray(Wk[:, sl]).astype(bf),
            "wv": np.ascontiguousarray(Wv[:, sl]).astype(bf),
            "wo": np.ascontiguousarray(Wo[sl, :]).astype(bf),
            "bqs": (bq[sl] * 0.125).reshape(P, 1).astype(np.float32),
            "bkc": bk[sl].reshape(P, 1).astype(np.float32),
            "bvr": bv[sl].reshape(1, P).astype(bf),
            "ident": identity,
        })
    return in_maps


def assemble(results, bo):
    bo = np.asarray(bo, np.float32)
    out = np.zeros((2, L, D), np.float32)
    for c in range(8):
        out[c // 4] += np.asarray(results[c]["out"], np.float32)
    out += bo[None, None, :]
    return out


def kernel(q, k, v, Wq, bq, Wk, bk, Wv, bv, Wo, bo):
    nc = get_nc()
    in_maps = make_in_maps(q, k, v, Wq, bq, Wk, bk, Wv, bv, Wo, bo)
    res = bass_utils.run_bass_kernel_spmd(nc, in_maps, core_ids=list(range(8)))
    return assemble(res.results, bo)


if __name__ == "__main__":
    build()
    print("build ok")
